# revision 57
# baseline (speedup 1.0000x reference)
"""Trainium2 Bass kernel for nn_MemKDMClassModel (retrieval_knn).

Computation (per sample b, fully data-parallel over the batch):
    d2[b,i]   = ||x_enc[b] - x_neigh[b,i]||^2
    w[b,i]    = exp(-d2[b,i] / sigma^2)          (= k^2 with k the RBF kernel)
    probs[b,c]= sum_i w[b,i]*onehot(y[b,i])[c] / (sum_i w[b,i] + EPS)

Sharding: pure data parallel - batch split across 8 NeuronCores.

Per-core mapping (512 samples/core, 4 blocks of 128 samples on partitions).
The stream of x_neigh (128 MiB/core) bounds runtime at ~378 us on the DMA
engines; every compute engine is kept below that pace:

  - per group of 8 comps: 6 "P" comps via PE fp32r eye-matmuls
    (diff = I@n + I@(-x), PSUM) + ACT Square-with-accum -> true d2 column;
    2 "V" comps via 2 DVE STT reductions -> t = -2*x.n and n2 columns.
  - w = exp(cvec*d2) on ACT; V columns use the per-partition activation
    bias (= -x2/sigma^2) so x2 never needs a per-comp add.
  - label scatter: blocks 0-2 use a per-class reduction
    probs[:,c] = sum_i (y_i==c)*w_i  (one DVE STT+accum per class, 100 ops)
    which overlaps the next block's stream; the last block scatters
    per-comp incrementally (split DVE/Pool) so the tail after the final
    DMA byte stays ~6 us.
  - rowsum is recovered as reduce_sum(probs) (== sum w) instead of
    accumulating it chunk-wise.
"""

import numpy as np

BS, N_COMP, ENC, DIM_Y = 4096, 128, 512, 100
EPS = 1e-10
N_CORES = 8
BS_L = BS // N_CORES          # 512 samples per core
BLK = 128                     # samples per block (partition dim)
NBLK = BS_L // BLK            # 4 blocks per core
G = 8                         # comps per DMA transfer (2 MiB each)
NG = N_COMP // G              # 16 DMA groups per block
PG = 6                        # P-path comps per group (j < PG)
VG = G - PG                   # V-path comps per group

_CACHE: dict = {}


def _build_nc():
    import concourse.bacc as bacc
    import concourse.tile as tile
    import concourse.mybir as mybir
    from concourse import bass

    f32 = mybir.dt.float32
    f32r = mybir.dt.float32r
    i32 = mybir.dt.int32
    AF = mybir.ActivationFunctionType
    ALU = mybir.AluOpType
    AX = mybir.AxisListType

    nc = bacc.Bacc("TRN2", target_bir_lowering=False, debug=False,
                   num_devices=N_CORES)

    # x_neigh and eye are declared float32r so the PE can run single-pass
    # (4x faster than fp32) matmuls on the streamed data; the BIR verifier
    # requires fp32r matmul operands to come from fp32r-typed producers.
    # Non-matmul consumers bitcast back to plain f32.
    x_dram = nc.dram_tensor("x_enc", [BS_L, ENC], f32, kind="ExternalInput")
    n_dram = nc.dram_tensor("x_neigh", [BS_L, N_COMP, ENC], f32r,
                            kind="ExternalInput")
    s_dram = nc.dram_tensor("sigma", [1, 1], f32, kind="ExternalInput")
    y_dram = nc.dram_tensor("y_neigh", [BS_L, N_COMP], i32,
                            kind="ExternalInput")
    # padded to 128 cols so DMA lines are 512 B (sub-512B descriptors pay a
    # 2x latency penalty); the host slices [:, :DIM_Y]
    OUT_W = 128
    out_dram = nc.dram_tensor("out", [BS_L, OUT_W], f32,
                              kind="ExternalOutput")

    with tile.TileContext(nc) as tc:
        with (
            tc.tile_pool(name="const", bufs=1) as constp,
            tc.tile_pool(name="neigh", bufs=8) as neighp,
            tc.tile_pool(name="last", bufs=1) as lastp,
            tc.tile_pool(name="xp", bufs=2) as xp,
            tc.tile_pool(name="small", bufs=3) as smallp,
            tc.tile_pool(name="dw", bufs=2) as dwp,
            tc.tile_pool(name="vstg", bufs=2) as vstg,
            tc.tile_pool(name="scr", bufs=2) as scrp,
            tc.tile_pool(name="oh", bufs=8) as ohp,
            tc.tile_pool(name="prob", bufs=3) as probp,
            tc.tile_pool(name="pdiff", bufs=6, space=bass.MemorySpace.PSUM) as pdiff,
            tc.tile_pool(name="pscratch", bufs=1, space=bass.MemorySpace.PSUM) as pscratch,
            tc.tile_pool(name="pmisc", bufs=1, space=bass.MemorySpace.PSUM) as pmisc,
        ):
            # ---- head: first neigh group leads on the SP queue so the
            # 360 GB/s DMA pipe fills immediately; everything else rides the
            # ACT queue whose DGE latency overlaps the first big transfer ----
            nt0 = neighp.tile([BLK, G * ENC], f32r, tag="ntile")
            nc.sync.dma_start(nt0[:], n_dram[0:BLK, 0:G, :])
            x0_tile = xp.tile([BLK, ENC], f32, tag="x")
            nc.scalar.dma_start(x0_tile[:], x_dram[0:BLK, :])
            sig = constp.tile([1, 1], f32)
            nc.scalar.dma_start(sig[:], s_dram[:])
            # eye and iota are generated on-chip by the (idle) Pool engine:
            # keeps their bytes off the DMA pipe entirely
            eyegen = constp.tile([128, 128], i32)
            nc.gpsimd.iota(eyegen[:], pattern=[[1, 128]], base=0,
                           channel_multiplier=-1)
            eye = constp.tile([128, 128], f32r)
            nc.gpsimd.tensor_scalar(eye[:], eyegen[:], 0, None,
                                    op0=ALU.is_equal)
            iotagen = constp.tile([128, DIM_Y], i32)
            nc.gpsimd.iota(iotagen[:], pattern=[[1, DIM_Y]], base=0,
                           channel_multiplier=0)
            iota = constp.tile([128, DIM_Y], f32)
            nc.gpsimd.tensor_copy(iota[:], iotagen[:])

            # ---- cvec = -1/sigma^2 broadcast to [128, 1] ----
            sig2 = constp.tile([1, 1], f32)
            nc.vector.tensor_scalar(sig2[:], sig[:], sig[0:1, 0:1], None,
                                    op0=ALU.mult)
            rsig2 = constp.tile([1, 1], f32)
            nc.vector.reciprocal(rsig2[:], sig2[:])
            nrsig2 = constp.tile([1, 1], f32)
            nc.vector.tensor_scalar_mul(nrsig2[:], rsig2[:], -1.0)
            ones_row = constp.tile([1, 128], f32)
            nc.vector.memset(ones_row[:], 1.0)
            cvec_ps = pmisc.tile([128, 1], f32)
            nc.tensor.matmul(cvec_ps[:], ones_row[:], nrsig2[:],
                             start=True, stop=True)
            cvec = constp.tile([128, 1], f32)
            nc.vector.tensor_copy(cvec[:], cvec_ps[:])

            sq_scratch = pscratch.tile([128, ENC], f32)
            ttr_scratch = constp.tile([128, ENC], f32)
            pool_scratch = constp.tile([128, ENC], f32)
            pool2_scratch = constp.tile([128, ENC], f32)
            eyer = eye[:]

            for b in range(NBLK):
                s0 = b * BLK
                last = (b == NBLK - 1)

                # ---- per-block inputs ----
                if b == 0:
                    x_tile = x0_tile
                else:
                    x_tile = xp.tile([BLK, ENC], f32, tag="x")
                    nc.sync.dma_start(x_tile[:], x_dram[s0:s0 + BLK, :])
                negx = xp.tile([BLK, ENC], f32r, tag="negx")
                nc.vector.tensor_scalar_mul(negx[:], x_tile[:], -1.0)
                negxr = negx[:]
                x2col = smallp.tile([BLK, 1], f32, tag="x2")
                nc.scalar.activation(sq_scratch[:], x_tile[:], AF.Square,
                                     accum_out=x2col[:, 0:1])
                # biasv = -x2/sigma^2 (per-partition bias for the V-col exp)
                biasv = smallp.tile([BLK, 1], f32, tag="biasv")
                nc.vector.tensor_scalar(biasv[:], x2col[:], cvec[:, 0:1],
                                        None, op0=ALU.mult)

                y_tile = smallp.tile([BLK, N_COMP], i32, tag="y")
                nc.scalar.dma_start(y_tile[:], y_dram[s0:s0 + BLK, :])
                y_f32 = smallp.tile([BLK, N_COMP], f32, tag="yf")
                nc.vector.tensor_copy(y_f32[:], y_tile[:])

                d2 = dwp.tile([BLK, N_COMP], f32, tag="d2")
                w = dwp.tile([BLK, N_COMP], f32, tag="w")
                tt = vstg.tile([BLK, NG * VG], f32, tag="tt")
                nn = vstg.tile([BLK, NG * VG], f32, tag="nn")
                d2r = d2[:].rearrange("p (g j) -> p g j", j=G)
                wr = w[:].rearrange("p (g j) -> p g j", j=G)
                ttr = tt[:].rearrange("p (g k) -> p g k", k=VG)
                nnr = nn[:].rearrange("p (g k) -> p g k", k=VG)

                if last:
                    probs_d = probp.tile([BLK, DIM_Y], f32, tag="pd")
                    probs_p = probp.tile([BLK, DIM_Y], f32, tag="pp")
                    first_sc = {"d": True, "p": True}
                    rs3 = smallp.tile([BLK, 1], f32, tag="rs3")
                    first_rs = [True]
                    rinv_holder = [None]
                    # staging for the tail groups' 3 V comps each
                    tts = vstg.tile([BLK, 12], f32, tag="tts")
                    nns = vstg.tile([BLK, 12], f32, tag="nns")
                else:
                    probs = probp.tile([BLK, DIM_Y], f32, tag="probs")

                def stt_v(nsl, g, j, eng=None, scratch=None):
                    nsl = nsl.bitcast(f32)
                    eng = eng or nc.vector
                    scratch = scratch or ttr_scratch
                    v = g * VG + (j - PG)
                    eng.scalar_tensor_tensor(
                        scratch[:], nsl, -2.0, x_tile[:],
                        op0=ALU.mult, op1=ALU.mult,
                        accum_out=tt[:, v:v + 1])
                    eng.scalar_tensor_tensor(
                        scratch[:], nsl, 1.0, nsl,
                        op0=ALU.mult, op1=ALU.mult,
                        accum_out=nn[:, v:v + 1])

                def pe_diff(nsl, g, j):
                    dtile = pdiff.tile([BLK, ENC], f32)
                    nc.tensor.matmul(dtile[:], eyer, nsl,
                                     start=True, stop=False)
                    nc.tensor.matmul(dtile[:], eyer, negxr,
                                     start=False, stop=True)
                    return dtile

                def scatter_comp(i, eng_key):
                    """Incremental per-comp scatter (last block only)."""
                    eng = nc.vector if eng_key == "d" else nc.gpsimd
                    tgt = probs_d if eng_key == "d" else probs_p
                    if first_sc[eng_key]:
                        eng.tensor_scalar(tgt[:], iota[:],
                                          y_f32[:, i:i + 1], w[:, i:i + 1],
                                          op0=ALU.is_equal, op1=ALU.mult)
                        first_sc[eng_key] = False
                        return
                    oh = ohp.tile([BLK, DIM_Y], f32, tag="oh" + eng_key)
                    eng.tensor_scalar(oh[:], iota[:], y_f32[:, i:i + 1],
                                      w[:, i:i + 1],
                                      op0=ALU.is_equal, op1=ALU.mult)
                    eng.tensor_tensor(tgt[:], tgt[:], oh[:], op=ALU.add)

                def window_exp(g0, g1):
                    """w cols for groups [g0, g1) <- exp."""
                    nc.vector.tensor_tensor(d2r[:, g0:g1, PG:G],
                                            ttr[:, g0:g1, :],
                                            nnr[:, g0:g1, :], op=ALU.add)
                    nc.scalar.activation(wr[:, g0:g1, 0:PG],
                                         d2r[:, g0:g1, 0:PG], AF.Exp,
                                         scale=cvec[:, 0:1])
                    nc.scalar.activation(wr[:, g0:g1, PG:G],
                                         d2r[:, g0:g1, PG:G], AF.Exp,
                                         scale=cvec[:, 0:1],
                                         bias=biasv[:, 0:1])

                def rs3_accum(c0, c1, eng=None):
                    """Accumulate rowsum over w cols [c0, c1) (last block).
                    Always on DVE: Pool has no free-axis reduce support in
                    neuronxcc."""
                    if first_rs[0]:
                        nc.vector.reduce_sum(rs3[:], w[:, c0:c1], axis=AX.X)
                        first_rs[0] = False
                        return
                    rpart = smallp.tile([BLK, 1], f32, tag="rspart")
                    nc.vector.reduce_sum(rpart[:], w[:, c0:c1], axis=AX.X)
                    nc.vector.tensor_tensor(rs3[:], rs3[:], rpart[:],
                                            op=ALU.add)

                # ---- main stream ----
                # Last block: groups >= TAIL_G0 keep the steady-state 6P/2V
                # engine mix but run per-pair epilogues (exp + rowsum +
                # scatter right after each pair of comps finishes) so every
                # dependency is local and each engine's in-order queue stays
                # shallow entering the tail; the last two groups stream as
                # per-comp singleton DMAs (V comps first) to stagger
                # arrivals.
                TAIL_G0 = NG - 4

                def emit_singles(g):
                    """Singleton-DMA group with a 5P/3V mix: the 3 V comps
                    stream first (one on Pool, two on DVE) so ACT only
                    carries 5 squares + small exps per 5.8us window and
                    enters the final chain with an empty queue."""
                    base = g * G
                    sb3 = (g - TAIL_G0) * 3
                    lastg = g == NG - 1
                    for j in [5, 6, 7, 0, 1, 2, 3, 4]:
                        i = base + j
                        t1 = lastp.tile([BLK, ENC], f32r,
                                        tag="l%d" % (i % 16))
                        nc.sync.dma_start(t1[:], n_dram[s0:s0 + BLK, i, :])
                        if j >= 5:
                            eng = nc.vector
                            scr5 = ttr_scratch
                            nslf = t1[:].bitcast(f32)
                            c5 = sb3 + j - 5
                            eng.scalar_tensor_tensor(
                                scr5[:], nslf, -2.0, x_tile[:],
                                op0=ALU.mult, op1=ALU.mult,
                                accum_out=tts[:, c5:c5 + 1])
                            eng.scalar_tensor_tensor(
                                scr5[:], nslf, 1.0, nslf,
                                op0=ALU.mult, op1=ALU.mult,
                                accum_out=nns[:, c5:c5 + 1])
                            if j == 7:
                                nc.vector.tensor_tensor(
                                    d2[:, base + 5:base + 8],
                                    tts[:, sb3:sb3 + 3],
                                    nns[:, sb3:sb3 + 3], op=ALU.add)
                                nc.scalar.activation(
                                    w[:, base + 5:base + 8],
                                    d2[:, base + 5:base + 8], AF.Exp,
                                    scale=cvec[:, 0:1], bias=biasv[:, 0:1])
                                rs3_accum(base + 5, base + 8, eng=nc.gpsimd)
                                scatter_comp(base + 5, "d")
                                scatter_comp(base + 6, "p")
                                scatter_comp(base + 7, "d")
                        else:
                            dtile = pe_diff(t1[:], g, j)
                            nc.scalar.activation(
                                sq_scratch[:], dtile[:], AF.Square,
                                accum_out=d2[:, i:i + 1])
                            if j in (1, 3):
                                nc.scalar.activation(
                                    w[:, i - 1:i + 1], d2[:, i - 1:i + 1],
                                    AF.Exp, scale=cvec[:, 0:1])
                                rs3_accum(i - 1, i + 1, eng=nc.gpsimd)
                                scatter_comp(i - 1, "p")
                                scatter_comp(i, "d")
                                if j == 3 and lastg:
                                    # probs_p final: fold in before the very
                                    # last comp's scatter
                                    nc.vector.tensor_tensor(
                                        probs_d[:], probs_d[:], probs_p[:],
                                        op=ALU.add)
                            elif j == 4:
                                # the final comp: minimal chain
                                nc.scalar.activation(
                                    w[:, i:i + 1], d2[:, i:i + 1],
                                    AF.Exp, scale=cvec[:, 0:1])
                                rs3_accum(i, i + 1,
                                          eng=None if lastg else nc.gpsimd)
                                if lastg:
                                    rs_eps = smallp.tile([BLK, 1], f32,
                                                         tag="rse")
                                    nc.vector.tensor_scalar_add(
                                        rs_eps[:], rs3[:], EPS)
                                    rinv = smallp.tile([BLK, 1], f32,
                                                       tag="rinv")
                                    nc.vector.reciprocal(rinv[:], rs_eps[:])
                                    rinv_holder[0] = rinv
                                scatter_comp(i, "d")

                for g in range(NG):
                    tailg = last and g >= TAIL_G0
                    singles = last and g >= NG - 2
                    if singles:
                        emit_singles(g)
                        continue
                    if b == 0 and g == 0:
                        ntile = nt0
                    else:
                        ntile = neighp.tile([BLK, G * ENC], f32r, tag="ntile")
                        nc.sync.dma_start(
                            ntile[:],
                            n_dram[s0:s0 + BLK, g * G:(g + 1) * G, :])
                    for j in range(G):
                        i = g * G + j
                        nsl = ntile[:, j * ENC:(j + 1) * ENC]
                        if j < PG:
                            dtile = pe_diff(nsl, g, j)
                            nc.scalar.activation(
                                sq_scratch[:], dtile[:], AF.Square,
                                accum_out=d2[:, i:i + 1])
                            if tailg and j == PG - 1:
                                # per-group P epilogue for groups 12/13
                                nc.scalar.activation(
                                    w[:, i - PG + 1:i + 1],
                                    d2[:, i - PG + 1:i + 1],
                                    AF.Exp, scale=cvec[:, 0:1])
                                for k in range(i - PG + 1, i + 1):
                                    scatter_comp(
                                        k, "d" if k % 2 == 0 else "p")
                        else:
                            stt_v(nsl, g, j)
                            if tailg and j == G - 1:
                                # V-pair epilogue (bias folds in x2)
                                nc.vector.tensor_tensor(
                                    d2r[:, g:g + 1, PG:G],
                                    ttr[:, g:g + 1, :],
                                    nnr[:, g:g + 1, :], op=ALU.add)
                                nc.scalar.activation(
                                    wr[:, g:g + 1, PG:G],
                                    d2r[:, g:g + 1, PG:G], AF.Exp,
                                    scale=cvec[:, 0:1], bias=biasv[:, 0:1])
                                # V comes last in natural order: one rowsum
                                # accum covers the whole group
                                rs3_accum(i - 7, i + 1)
                                scatter_comp(i - 1, "d")
                                scatter_comp(i, "p")
                    if last and not tailg and g % 2 == 1:
                        # 2-group window epilogue for the earlier groups
                        window_exp(g - 1, g + 1)
                        rs3_accum((g - 1) * G, (g + 1) * G)
                        for i in range((g - 1) * G, (g + 1) * G):
                            scatter_comp(i, "d" if i % 2 == 0 else "p")

                if not last:
                    # ---- block epilogue: exp over all cols, then the
                    # per-class scatter (overlaps the next block's stream) ---
                    window_exp(0, NG)
                    for c in range(DIM_Y):
                        scr = scrp.tile([BLK, N_COMP], f32, tag="scr")
                        nc.vector.scalar_tensor_tensor(
                            scr[:], y_f32[:], float(c), w[:],
                            op0=ALU.is_equal, op1=ALU.mult,
                            accum_out=probs[:, c:c + 1])
                    pfin = probs
                    rowsum = smallp.tile([BLK, 1], f32, tag="rs")
                    nc.vector.reduce_sum(rowsum[:], pfin[:], axis=AX.X)
                    rs_eps = smallp.tile([BLK, 1], f32, tag="rse")
                    nc.vector.tensor_scalar_add(rs_eps[:], rowsum[:], EPS)
                    rinv = smallp.tile([BLK, 1], f32, tag="rinv")
                    nc.vector.reciprocal(rinv[:], rs_eps[:])
                else:
                    # merge + eps + recip were already emitted in the tail
                    pfin = probs_d
                    rinv = rinv_holder[0]

                # ---- normalize + store ----
                out_sb = probp.tile([BLK, OUT_W], f32, tag="out")
                nc.vector.memset(out_sb[:, DIM_Y:OUT_W], 0.0)
                nc.vector.tensor_scalar(out_sb[:, 0:DIM_Y], pfin[:],
                                        rinv[:, 0:1], None, op0=ALU.mult)
                if last:
                    # SP queue is idle at the end; HWDGE path, no SWDGE prep
                    nc.sync.dma_start(out_dram[s0:s0 + BLK, :], out_sb[:])
                else:
                    nc.gpsimd.dma_start(out_dram[s0:s0 + BLK, :], out_sb[:])

    nc.compile()
    return nc


def _get_nc():
    if "nc" not in _CACHE:
        _CACHE["nc"] = _build_nc()
    return _CACHE["nc"]


def _get_exec():
    """Build (once) a jitted shard_map executable over 8 cores.

    Returns (fn, in_names, out_names, out_avals, n_params, mesh).
    Call as fn(*concat_inputs, *concat_zero_outputs); outputs donated.
    """
    if "exec" in _CACHE:
        return _CACHE["exec"]
    import jax
    import concourse.mybir as mybir
    from jax.sharding import Mesh, PartitionSpec
    from jax.experimental.shard_map import shard_map
    from concourse.bass2jax import (_bass_exec_p, install_neuronx_cc_hook,
                                    partition_id_tensor)

    install_neuronx_cc_hook()
    nc = _get_nc()
    partition_name = (nc.partition_id_tensor.name
                      if nc.partition_id_tensor else None)
    in_names, out_names, out_avals = [], [], []
    for alloc in nc.m.functions[0].allocations:
        if not isinstance(alloc, mybir.MemoryLocationSet):
            continue
        name = alloc.memorylocations[0].name
        if alloc.kind == "ExternalInput":
            if name != partition_name:
                in_names.append(name)
        elif alloc.kind == "ExternalOutput":
            out_names.append(name)
            out_avals.append(jax.core.ShapedArray(
                tuple(alloc.tensor_shape), mybir.dt.np(alloc.dtype)))
    n_params = len(in_names)
    all_in_names = in_names + out_names
    if partition_name is not None:
        all_in_names = all_in_names + [partition_name]
    donate = tuple(range(n_params, n_params + len(out_names)))

    def _body(*args):
        operands = list(args)
        if partition_name is not None:
            operands.append(partition_id_tensor())
        outs = _bass_exec_p.bind(
            *operands,
            out_avals=tuple(out_avals),
            in_names=tuple(all_in_names),
            out_names=tuple(out_names),
            lowering_input_output_aliases=(),
            sim_require_finite=True,
            sim_require_nnan=True,
            nc=nc,
        )
        return tuple(outs)

    devices = jax.devices()[:N_CORES]
    mesh = Mesh(np.asarray(devices), ("core",))
    specs = (PartitionSpec("core"),) * (n_params + len(out_names))
    out_specs = (PartitionSpec("core"),) * len(out_names)
    fn = jax.jit(
        shard_map(_body, mesh=mesh, in_specs=specs, out_specs=out_specs,
                  check_rep=False),
        donate_argnums=donate, keep_unused=True)
    _CACHE["exec"] = (fn, in_names, out_names, out_avals, n_params, mesh)
    return _CACHE["exec"]


def _concat_inputs(x_enc, x_neigh, sig, y_neigh_i32):
    """Per-input concatenation over cores, ordered by the NEFF's in_names."""
    per_core = {
        "x_enc": lambda c: x_enc[c * BS_L:(c + 1) * BS_L],
        "x_neigh": lambda c: x_neigh[c * BS_L:(c + 1) * BS_L],
        "sigma": lambda c: sig,
        "y_neigh": lambda c: y_neigh_i32[c * BS_L:(c + 1) * BS_L],
    }
    _, in_names, _, _, _, _ = _get_exec()
    return [np.concatenate([per_core[name](c) for c in range(N_CORES)], axis=0)
            for name in in_names]


def _zero_outs():
    _, _, _, out_avals, _, _ = _get_exec()
    return [np.zeros((N_CORES * a.shape[0], *a.shape[1:]), a.dtype)
            for a in out_avals]


def kernel(x_enc, x_neigh, sigma, y_neigh):
    x_enc = np.ascontiguousarray(np.asarray(x_enc, dtype=np.float32))
    x_neigh = np.ascontiguousarray(np.asarray(x_neigh, dtype=np.float32))
    sig = np.ascontiguousarray(np.asarray(sigma).astype(np.float32).reshape(1, 1))
    y_neigh_i32 = np.ascontiguousarray(np.asarray(y_neigh).astype(np.int32))

    fn, in_names, out_names, out_avals, n_params, mesh = _get_exec()
    concat_in = _concat_inputs(x_enc, x_neigh, sig, y_neigh_i32)
    out_arrs = fn(*concat_in, *_zero_outs())
    oi = out_names.index("out")
    out = np.asarray(out_arrs[oi]).reshape(N_CORES * BS_L, -1)
    return np.ascontiguousarray(out[:, :DIM_Y]).astype(np.float32)


if __name__ == "__main__":
    rng = np.random.default_rng(0)
    x_enc = rng.standard_normal((BS, ENC), dtype=np.float32)
    x_neigh = rng.standard_normal((BS, N_COMP, ENC), dtype=np.float32)
    sigma = 20.0 * np.ones((1,), dtype=np.float32)  # large: exercises nonzero path
    y_neigh = rng.integers(0, DIM_Y, size=(BS, N_COMP)).astype(np.int32)
    out = kernel(x_enc=x_enc, x_neigh=x_neigh, sigma=sigma, y_neigh=y_neigh)
    # numpy oracle
    d2 = np.maximum(
        (x_enc ** 2).sum(-1)[:, None]
        + (x_neigh ** 2).sum(-1)
        - 2.0 * np.einsum("bd,bnd->bn", x_enc, x_neigh), 0.0)
    w = np.exp(-d2 / (sigma[0] ** 2))
    probs = np.zeros((BS, DIM_Y), np.float32)
    np.add.at(probs, (np.arange(BS)[:, None], y_neigh), w.astype(np.float32))
    probs /= (w.sum(-1, keepdims=True).astype(np.float32) + EPS)
    print("max abs diff:", np.abs(out - probs).max())
    print("rel err:", np.linalg.norm(out - probs) / np.linalg.norm(probs))
    print("ref max:", probs.max(), "out max:", out.max())
    print("out nonzero:", np.count_nonzero(out), "/", out.size)
